# revision 2
# baseline (speedup 1.0000x reference)
"""GCN link-prediction kernel for 8 Trainium2 NeuronCores (bf16, batched gathers).

Strategy (dst-node sharding, edge aggregation via one-hot matmul):
  - Nodes (dst) sharded across 8 cores (12500 each); each core processes the
    edges whose dst lands in its shard (plus its self-loops).
  - GCN sym-norm factorizes per node: out[d] = dinv[d] * sum_{s in N(d)+d}
    dinv[s]*h[s].  The dinv[s] pre-scale is folded into the dense input on the
    host (layer 1) or into the previous layer's output scaling (layer 2), so
    htab rows need no extra scaling on device.
  - Dense phase: tblT [128, N] bf16 (transposed, dinv-prescaled) streamed in
    1024-col slabs; per 128-row tile one bf16 matmul (lhsT=tblT slice, rhs=W);
    PSUM copied+cast to bf16 on the scalar engine; written to DRAM htab [N,128]
    in 1024-row batched DMAs.
  - Aggregation: edges sorted by dst block (128 nodes), packed 128-per-chunk;
    per dst block ONE batched indirect DMA gathers all kb*128 source rows
    (bf16, 256B descriptors); the one-hot selection matrix for all kb chunks is
    built with ONE broadcast is_equal on DVE; kb bf16 matmuls accumulate into
    PSUM; block epilogue applies dinv/bias/relu/second-scale and emits both
    row-major (for decode) and transposed (for the next layer) outputs.
  - Decode: z [N,64] bf16; per 64-chunk group one batched indirect gather per
    side (128B descriptors), bf16 multiply, segmented tensor_reduce.
Host does index-only prep (degree, sorting, padding, transpose/cast of x) and
inter-program concatenation of shards.
"""
import numpy as np
from ml_dtypes import bfloat16

import concourse.bass as bass
import concourse.bacc as bacc
import concourse.mybir as mybir
import concourse.tile as tile
from concourse.bass_utils import run_bass_kernel_spmd
from concourse.masks import make_identity

f32 = mybir.dt.float32
bf16 = mybir.dt.bfloat16
i32 = mybir.dt.int32

N = 100000
E = 1600000
EL = 1048576
IN = 128
HID = 128
OUT = 64
NCORES = 8
NPC = N // NCORES          # 12500 nodes per core
NBLK = (NPC + 127) // 128  # 98 dst blocks per core
P = 128

_prog_cache = {}


def _prep(edge_index, n=N, ncores=NCORES):
    npc = n // ncores
    nblk = (npc + 127) // 128
    src = np.asarray(edge_index[0], dtype=np.int64)
    dst = np.asarray(edge_index[1], dtype=np.int64)
    deg = np.bincount(dst, minlength=n).astype(np.float64) + 1.0
    dinv = (1.0 / np.sqrt(deg)).astype(np.float32)

    # per (core, block) edge lists, self-loops included
    core_of = dst // npc
    per_core = []
    counts = np.zeros((ncores, nblk), dtype=np.int64)
    for c in range(ncores):
        m = core_of == c
        s_c = src[m]
        d_c = dst[m] - c * npc
        loop = np.arange(npc, dtype=np.int64)
        s_c = np.concatenate([s_c, loop + c * npc])
        d_c = np.concatenate([d_c, loop])
        blk = d_c // 128
        order = np.argsort(blk, kind="stable")
        s_c, d_c, blk = s_c[order], d_c[order], blk[order]
        per_core.append((s_c, d_c, blk))
        counts[c] = np.bincount(blk, minlength=nblk)

    kb = ((counts.max(axis=0) + 127) // 128).astype(np.int64)  # chunks per blk
    ktot = int(kb.sum())
    col_off = np.concatenate([[0], np.cumsum(kb)[:-1]])

    srcs = np.zeros((ncores, 128, ktot), dtype=np.int32)
    dstl = np.full((ncores, 128, ktot), 999.0, dtype=np.float32)
    for c in range(ncores):
        s_c, d_c, blk = per_core[c]
        pos = 0
        for b in range(nblk):
            m = int(counts[c, b])
            cols = int(kb[b])
            spad = np.zeros(cols * 128, dtype=np.int32)
            dpad = np.full(cols * 128, 999.0, dtype=np.float32)
            spad[:m] = s_c[pos:pos + m]
            dpad[:m] = (d_c[pos:pos + m] % 128).astype(np.float32)
            srcs[c, :, col_off[b]:col_off[b] + cols] = spad.reshape(cols, 128).T
            dstl[c, :, col_off[b]:col_off[b] + cols] = dpad.reshape(cols, 128).T
            pos += m

    # dinv arranged per dst block [128, nblk]
    dinv_dst = np.ones((ncores, 128, nblk), dtype=np.float32)
    for c in range(ncores):
        v = dinv[c * npc:(c + 1) * npc]
        vp = np.ones(nblk * 128, dtype=np.float32)
        vp[:npc] = v
        dinv_dst[c] = vp.reshape(nblk, 128).T

    return dict(srcs=srcs, dstl=dstl, kb=kb, col_off=col_off,
                dinv_dst=dinv_dst, dinv=dinv, ktot=ktot)


def _build_layer(kb, col_off, n=N, npc=NPC, num_devices=NCORES, reps=1):
    """One GCN layer: tblT [128, n] bf16 -> out_rows [npc,128] + outT [128,npc]."""
    nblk = (npc + 127) // 128
    nt = (n + 127) // 128
    ktot = int(kb.sum())
    nc = bacc.Bacc("TRN2", target_bir_lowering=False, debug=False,
                   num_devices=num_devices)
    tblT = nc.dram_tensor("tblT", [P, n], bf16, kind="ExternalInput").ap()
    W = nc.dram_tensor("W", [P, P], bf16, kind="ExternalInput").ap()
    brep = nc.dram_tensor("brep", [P, P], f32, kind="ExternalInput").ap()
    thr = nc.dram_tensor("thr", [P, 1], f32, kind="ExternalInput").ap()
    iota = nc.dram_tensor("iota", [P, P], f32, kind="ExternalInput").ap()
    srcs = nc.dram_tensor("srcs", [P, ktot], i32, kind="ExternalInput").ap()
    dstl = nc.dram_tensor("dstl", [P, ktot], f32, kind="ExternalInput").ap()
    dd = nc.dram_tensor("dd", [P, nblk], f32, kind="ExternalInput").ap()
    sc2 = nc.dram_tensor("sc2", [P, nblk], f32, kind="ExternalInput").ap()
    out_rows = nc.dram_tensor("out_rows", [npc, P], bf16,
                              kind="ExternalOutput").ap()
    outT = nc.dram_tensor("outT", [P, npc], bf16, kind="ExternalOutput").ap()

    WG = 8  # row tiles per htab write slab / outT col blocks per write slab
    with tile.TileContext(nc) as tc:
        with (tc.tile_pool(name="const", bufs=1) as cpool,
              tc.tile_pool(name="xin", bufs=3) as xpool,
              tc.tile_pool(name="hs", bufs=3) as hpool,
              tc.tile_pool(name="g", bufs=4) as gpool,
              tc.tile_pool(name="m", bufs=4) as mpool,
              tc.tile_pool(name="ob", bufs=4) as opool,
              tc.tile_pool(name="ot", bufs=2) as otpool,
              tc.tile_pool(name="dram", bufs=1, space="DRAM") as dpool):
            htab = dpool.tile([n, P], bf16, name="htab")
            W_t = cpool.tile([P, P], bf16, name="W_t")
            nc.sync.dma_start(out=W_t[:], in_=W[:])
            brep_t = cpool.tile([P, P], f32, name="brep_t")
            nc.sync.dma_start(out=brep_t[:], in_=brep[:])
            thr_t = cpool.tile([P, 1], f32, name="thr_t")
            nc.sync.dma_start(out=thr_t[:], in_=thr[:])
            iota_t = cpool.tile([P, P], f32, name="iota_t")
            nc.sync.dma_start(out=iota_t[:], in_=iota[:])
            srcs_t = cpool.tile([P, ktot], i32, name="srcs_t")
            nc.sync.dma_start(out=srcs_t[:], in_=srcs[:])
            dstl_t = cpool.tile([P, ktot], f32, name="dstl_t")
            nc.sync.dma_start(out=dstl_t[:], in_=dstl[:])
            dd_t = cpool.tile([P, nblk], f32, name="dd_t")
            nc.sync.dma_start(out=dd_t[:], in_=dd[:])
            sc2_t = cpool.tile([P, nblk], f32, name="sc2_t")
            nc.sync.dma_start(out=sc2_t[:], in_=sc2[:])
            ident = cpool.tile([P, P], bf16, name="ident")
            make_identity(nc, ident[:])

            rep_cm = tc.For_i(0, reps, 1) if reps > 1 else None
            if rep_cm is not None:
                rep_cm.__enter__()

            # ---- dense: htab = cast_bf16(tblT.T @ W), in 1024-row groups
            with tc.tile_pool(name="psA", bufs=3, space="PSUM") as psA:
                for g0 in range(0, nt, WG):
                    r0 = g0 * 128
                    grows = min(WG * 128, n - r0)
                    gtiles = (grows + 127) // 128
                    xg = xpool.tile([P, WG * P], bf16, name="xg", tag="xg")
                    nc.sync.dma_start(out=xg[:, :grows], in_=tblT[:, r0:r0+grows])
                    hs = hpool.tile([P, WG * P], bf16, name="hs", tag="hs")
                    for i in range(gtiles):
                        rows = min(128, grows - i * 128)
                        ps = psA.tile([P, P], f32, name="psA", tag="psA")
                        nc.tensor.matmul(ps[:rows, :],
                                         lhsT=xg[:, i*128:i*128+rows],
                                         rhs=W_t[:], start=True, stop=True)
                        nc.scalar.copy(out=hs[:rows, i*128:(i+1)*128],
                                       in_=ps[:rows, :])
                    ft = grows // 128
                    if ft:
                        nc.scalar.dma_start(
                            out=htab[r0:r0 + ft*128, :].rearrange(
                                "(g p) f -> p g f", p=P),
                            in_=hs[:, :ft*128].rearrange(
                                "p (g f) -> p g f", f=P))
                    rem = grows - ft * 128
                    if rem:
                        nc.scalar.dma_start(
                            out=htab[r0 + ft*128:r0 + grows, :],
                            in_=hs[:rem, ft*128:ft*128+P])

            # ---- aggregation: per dst block one batched gather + one-hot mm
            with tc.tile_pool(name="psB", bufs=4, space="PSUM") as psB, \
                 tc.tile_pool(name="psT", bufs=2, space="PSUM") as psTp:
                ot = None
                for b in range(nblk):
                    kbb = int(kb[b])
                    c0 = int(col_off[b])
                    rows = min(128, npc - b * 128)
                    g_t = gpool.tile([P, kbb * P], bf16, name="g", tag="g")
                    nc.gpsimd.indirect_dma_start(
                        out=g_t[:], out_offset=None, in_=htab[:],
                        in_offset=bass.IndirectOffsetOnAxis(
                            ap=srcs_t[:, c0:c0+kbb], axis=0))
                    M_t = mpool.tile([P, kbb * P], bf16, name="M", tag="M")
                    nc.vector.tensor_tensor(
                        out=M_t[:].rearrange("p (k f) -> p k f", f=P),
                        in0=iota_t[:].rearrange("p (o f) -> p o f", o=1
                                                ).broadcast_to([P, kbb, P]),
                        in1=dstl_t[:, c0:c0+kbb].rearrange(
                            "p (k o) -> p k o", o=1).broadcast_to([P, kbb, P]),
                        op=mybir.AluOpType.is_equal)
                    ps = psB.tile([P, P], f32, name="psB", tag="psB")
                    for k in range(kbb):
                        nc.tensor.matmul(ps[:], lhsT=M_t[:, k*P:(k+1)*P],
                                         rhs=g_t[:, k*P:(k+1)*P],
                                         start=(k == 0), stop=(k == kbb - 1))
                    ob = opool.tile([P, P], f32, name="ob", tag="ob")
                    nc.vector.tensor_scalar(
                        out=ob[:], in0=ps[:], scalar1=dd_t[:, b:b+1],
                        scalar2=None, op0=mybir.AluOpType.mult)
                    nc.vector.tensor_tensor(out=ob[:], in0=ob[:],
                                            in1=brep_t[:],
                                            op=mybir.AluOpType.add)
                    orow = opool.tile([P, P], bf16, name="orow", tag="orow")
                    nc.vector.tensor_scalar(
                        out=orow[:], in0=ob[:], scalar1=thr_t[:, :1],
                        scalar2=sc2_t[:, b:b+1], op0=mybir.AluOpType.max,
                        op1=mybir.AluOpType.mult)
                    nc.sync.dma_start(out=out_rows[b*128:b*128+rows, :],
                                      in_=orow[:rows, :])
                    # transposed copy for the next layer's dense input
                    if b % WG == 0:
                        ot = otpool.tile([P, WG * P], bf16, name="ot",
                                         tag="ot")
                    pst = psTp.tile([P, P], bf16, name="psT", tag="psT")
                    nc.tensor.transpose(out=pst[:, :rows], in_=orow[:rows, :],
                                        identity=ident[:rows, :rows])
                    off = (b % WG) * 128
                    nc.scalar.copy(out=ot[:, off:off+rows],
                                   in_=pst[:, :rows])
                    if b % WG == WG - 1 or b == nblk - 1:
                        b0 = b - (b % WG)
                        cw = b * 128 + rows - b0 * 128
                        nc.scalar.dma_start(out=outT[:, b0*128:b0*128+cw],
                                            in_=ot[:, :cw])

            if rep_cm is not None:
                rep_cm.__exit__(None, None, None)
    nc.compile()
    return nc


def _build_decode(n=N, cc=EL // NCORES // 128, num_devices=NCORES, reps=1):
    """Decode: o[p, c] = sum_f z[ii[p,c], f] * z[jj[p,c], f]."""
    F = OUT
    KG = 64  # chunks per gather group
    nc = bacc.Bacc("TRN2", target_bir_lowering=False, debug=False,
                   num_devices=num_devices)
    z = nc.dram_tensor("z", [n, F], bf16, kind="ExternalInput").ap()
    ii = nc.dram_tensor("ii", [P, cc], i32, kind="ExternalInput").ap()
    jj = nc.dram_tensor("jj", [P, cc], i32, kind="ExternalInput").ap()
    o = nc.dram_tensor("o", [P, cc], f32, kind="ExternalOutput").ap()
    with tile.TileContext(nc) as tc:
        with (tc.tile_pool(name="const", bufs=1) as cpool,
              tc.tile_pool(name="gath", bufs=3) as gpool,
              tc.tile_pool(name="pr", bufs=3) as ppool):
            ii_t = cpool.tile([P, cc], i32, name="ii_t")
            nc.sync.dma_start(out=ii_t[:], in_=ii[:])
            jj_t = cpool.tile([P, cc], i32, name="jj_t")
            nc.sync.dma_start(out=jj_t[:], in_=jj[:])
            oc = cpool.tile([P, cc], f32, name="oc")
            rep_cm = tc.For_i(0, reps, 1) if reps > 1 else None
            if rep_cm is not None:
                rep_cm.__enter__()
            for c0 in range(0, cc, KG):
                kg = min(KG, cc - c0)
                gi = gpool.tile([P, KG * F], bf16, name="gi", tag="gi")
                nc.gpsimd.indirect_dma_start(
                    out=gi[:, :kg*F], out_offset=None, in_=z[:],
                    in_offset=bass.IndirectOffsetOnAxis(
                        ap=ii_t[:, c0:c0+kg], axis=0))
                gj = gpool.tile([P, KG * F], bf16, name="gj", tag="gj")
                nc.gpsimd.indirect_dma_start(
                    out=gj[:, :kg*F], out_offset=None, in_=z[:],
                    in_offset=bass.IndirectOffsetOnAxis(
                        ap=jj_t[:, c0:c0+kg], axis=0))
                pr = ppool.tile([P, KG * F], bf16, name="pr", tag="pr")
                nc.vector.tensor_tensor(out=pr[:, :kg*F], in0=gi[:, :kg*F],
                                        in1=gj[:, :kg*F],
                                        op=mybir.AluOpType.mult)
                nc.vector.tensor_reduce(
                    out=oc[:, c0:c0+kg],
                    in_=pr[:, :kg*F].rearrange("p (k f) -> p k f", f=F),
                    axis=mybir.AxisListType.X, op=mybir.AluOpType.add)
            if rep_cm is not None:
                rep_cm.__exit__(None, None, None)
            nc.sync.dma_start(out=o[:], in_=oc[:])
    nc.compile()
    return nc


def _get_programs(meta):
    key = ("progs", meta["ktot"], tuple(meta["kb"].tolist()))
    if key not in _prog_cache:
        _prog_cache[key] = (_build_layer(meta["kb"], meta["col_off"]),
                            _build_decode())
    return _prog_cache[key]


def _layer_maps(meta, tblT, Wv, brv, thv, sc2v, iota, ncores=NCORES):
    return [
        {"tblT": tblT, "W": Wv, "brep": brv, "thr": thv, "iota": iota,
         "srcs": meta["srcs"][c], "dstl": meta["dstl"][c],
         "dd": meta["dinv_dst"][c],
         "sc2": meta["dinv_dst"][c] if sc2v is None else sc2v}
        for c in range(ncores)
    ]


def kernel(x, W1, b1, W2, b2, edge_index, edge_label_idx):
    x = np.asarray(x, dtype=np.float32)
    W1 = np.asarray(W1, dtype=np.float32)
    b1 = np.asarray(b1, dtype=np.float32)
    W2 = np.asarray(W2, dtype=np.float32)
    b2 = np.asarray(b2, dtype=np.float32)
    eidx = np.asarray(edge_index)
    eli = np.asarray(edge_label_idx)

    meta = _prep(eidx)
    nc_layer, nc_dec = _get_programs(meta)

    dinv = meta["dinv"]
    iota = np.broadcast_to(np.arange(P, dtype=np.float32)[None, :],
                           (P, P)).copy()
    xT1 = np.ascontiguousarray((x * dinv[:, None]).T).astype(bfloat16)
    W1b = W1.astype(bfloat16)
    W2p = np.zeros((P, P), np.float32)
    W2p[:, :OUT] = W2
    W2b = W2p.astype(bfloat16)
    b1rep = np.broadcast_to(b1[None, :], (P, P)).astype(np.float32).copy()
    b2rep = np.zeros((P, P), np.float32)
    b2rep[:, :OUT] = b2[None, :]
    thr_relu = np.zeros((P, 1), np.float32)
    thr_id = np.full((P, 1), -1e30, np.float32)
    ones_sc = np.ones((P, NBLK), np.float32)

    core_ids = list(range(NCORES))
    # layer 1: outputs h1s = dinv*relu(h1) transposed, bf16
    res1 = run_bass_kernel_spmd(
        nc_layer, _layer_maps(meta, xT1, W1b, b1rep, thr_relu, None, iota),
        core_ids)
    h1T = np.concatenate([res1.results[c]["outT"] for c in range(NCORES)],
                         axis=1)
    # layer 2: z rows (cols 64.. are exactly 0)
    res2 = run_bass_kernel_spmd(
        nc_layer, _layer_maps(meta, h1T, W2b, b2rep, thr_id, ones_sc, iota),
        core_ids)
    z = np.concatenate([res2.results[c]["out_rows"] for c in range(NCORES)],
                       axis=0)
    z64 = np.ascontiguousarray(z[:, :OUT])
    # decode
    PPC = EL // NCORES
    CC = PPC // 128
    dec_maps = []
    for c in range(NCORES):
        i0 = np.asarray(eli[0][c*PPC:(c+1)*PPC], dtype=np.int32)
        j0 = np.asarray(eli[1][c*PPC:(c+1)*PPC], dtype=np.int32)
        dec_maps.append({"z": z64,
                         "ii": i0.reshape(CC, 128).T.copy(),
                         "jj": j0.reshape(CC, 128).T.copy()})
    res3 = run_bass_kernel_spmd(nc_dec, dec_maps, core_ids)
    out = np.concatenate(
        [res3.results[c]["o"].T.reshape(-1) for c in range(NCORES)])
    return out.astype(np.float32)


# revision 6
# speedup vs baseline: 1.5538x; 1.5538x over previous
"""GCN link-prediction kernel for 8 Trainium2 NeuronCores (bf16, dma_gather).

Strategy (dst-node sharding, edge aggregation via one-hot matmul):
  - Nodes (dst) sharded across 8 cores (12500 each); each core processes the
    edges whose dst lands in its shard (plus its self-loops).
  - GCN sym-norm factorizes per node: out[d] = dinv[d] * sum_{s in N(d)+d}
    dinv[s]*h[s].  The dinv[s] pre-scale is folded into the dense input on the
    host (layer 1) or into the previous layer's output scaling (layer 2).
  - Dense phase: tblT [128, N] bf16 (transposed, prescaled) streamed in
    1024-col slabs; one bf16 matmul per 128-row tile (lhsT=tblT slice, rhs=W);
    PSUM cast-copied to bf16 on the scalar engine; 1024-row batched DMA writes
    to DRAM htab [N,128] bf16.
  - Aggregation: edges packed into a core-common slot stream grouped by
    (super-group of 8 dst blocks) x (source bank of 25000 rows).  Per segment
    ONE dma_gather (int16 bank-local indices) fetches all rows; the one-hot
    selection matrices for all (chunk, block) pairs are built with ONE
    broadcast is_equal on DVE; one bf16 matmul per pair accumulates into the
    block's PSUM tile; block epilogue applies dinv/bias/relu/second-scale and
    emits row-major (decode input) and transposed (next layer input) outputs.
  - Decode: z = layer-2 out_rows [N,128] bf16 (cols 64.. zero); candidate
    pairs grouped by (bank_i, bank_j) into <=8192-slot subcalls; two
    dma_gathers per subcall, bf16 product, segmented reduce over 64 features.
Host does index-only prep (degree, sorting, packing, transpose/cast of x) and
inter-program concatenation of shards.
"""
import numpy as np
from ml_dtypes import bfloat16

import concourse.bass as bass
import concourse.bacc as bacc
import concourse.mybir as mybir
import concourse.tile as tile
from concourse.bass_utils import run_bass_kernel_spmd
from concourse.masks import make_identity

f32 = mybir.dt.float32
bf16 = mybir.dt.bfloat16
i16 = mybir.dt.int16

N = 100000
E = 1600000
EL = 1048576
OUT = 64
NCORES = 8
NPC = N // NCORES          # 12500 nodes per core
NBLK = (NPC + 127) // 128  # 98 dst blocks per core
P = 128
BANKS = 4
BS = N // BANKS            # 25000 rows per gather bank (int16 range)
SG = 6                     # dst blocks per super-group (6 PSUM banks + 2)
DSUB = 8192                # decode slots per gather subcall

_prog_cache = {}


def _pack_idx16(flat):
    """Slot i -> partition i%16, col i//16; replicated to all 8 groups."""
    assert len(flat) % 16 == 0
    s = len(flat) // 16
    base = flat.reshape(s, 16).T.astype(np.int16)
    return np.tile(base, (8, 1)).copy()


def _prep(edge_index, n=N, ncores=NCORES, banks=BANKS, sg=SG):
    npc = n // ncores
    nblk = (npc + 127) // 128
    bs = n // banks
    nsg = (nblk + sg - 1) // sg
    src = np.asarray(edge_index[0], dtype=np.int64)
    dst = np.asarray(edge_index[1], dtype=np.int64)
    deg = np.bincount(dst, minlength=n).astype(np.float64) + 1.0
    dinv = (1.0 / np.sqrt(deg)).astype(np.float32)

    # per-core edge lists (self-loops included), sorted by (block, bank)
    core_of = dst // npc
    per_core = []
    cnt = np.zeros((ncores, nblk, banks), dtype=np.int64)
    for c in range(ncores):
        m = core_of == c
        loop = np.arange(npc, dtype=np.int64)
        s_c = np.concatenate([src[m], loop + c * npc])
        d_c = np.concatenate([dst[m] - c * npc, loop])
        blk = d_c // 128
        bank = s_c // bs
        order = np.lexsort((bank, blk))
        s_c, d_c, blk, bank = s_c[order], d_c[order], blk[order], bank[order]
        per_core.append((s_c, d_c))
        cnt[c] = np.bincount(blk * banks + bank,
                             minlength=nblk * banks).reshape(nblk, banks)

    cmax = cnt.max(axis=0)  # [nblk, banks] common slot count per (block,bank)

    # ---- common layout: stream of (sg, bank) segments, each padded to 128
    slot_block = []       # per slot: block id (or -1 pad)
    segs = []             # per (sg, bank): dict
    blk_off = np.zeros((nblk, banks), dtype=np.int64)  # slot pos of (b, k)
    pairs = []            # emission order: (seg_idx, ch_local, block)
    pos = 0
    for g in range(nsg):
        b0, b1 = g * sg, min((g + 1) * sg, nblk)
        for k in range(banks):
            seg_cnt = cmax[b0:b1, k]
            raw = int(seg_cnt.sum())
            slots = -(-raw // 128) * 128 if raw else 0
            sb = np.full(slots, -1, dtype=np.int64)
            p0 = 0
            for b in range(b0, b1):
                blk_off[b, k] = pos + p0
                sb[p0:p0 + cmax[b, k]] = b
                p0 += cmax[b, k]
            seg_pairs = []
            for ch in range(slots // 128):
                for b in np.unique(sb[ch * 128:(ch + 1) * 128]):
                    if b >= 0:
                        seg_pairs.append((ch, int(b)))
            segs.append(dict(sg=g, bank=k, slots=slots, slot0=pos,
                             pair0=len(pairs), npairs=len(seg_pairs),
                             pairs=seg_pairs))
            for ch, b in seg_pairs:
                pairs.append((len(segs) - 1, ch, b))
            slot_block.append(sb)
            pos += slots
    slots_tot = pos
    npairs = len(pairs)
    slot_block = (np.concatenate(slot_block) if slot_block
                  else np.zeros(0, np.int64))

    # start/stop flags per pair (first/last pair of each block in emission)
    first_of = {}
    last_of = {}
    for j, (_, _, b) in enumerate(pairs):
        if b not in first_of:
            first_of[b] = j
        last_of[b] = j
    pair_start = np.zeros(npairs, bool)
    pair_stop = np.zeros(npairs, bool)
    for b, j in first_of.items():
        pair_start[j] = True
    for b, j in last_of.items():
        pair_stop[j] = True

    # ---- per-core slot data
    idx16 = np.zeros((ncores, 128, slots_tot // 16), np.int16)
    dstl = np.full((ncores, 128, npairs), 999.0, np.float32)
    for c in range(ncores):
        s_c, d_c = per_core[c]
        sidx = np.zeros(slots_tot, np.int64)
        sdl = np.full(slots_tot, 999.0, np.float32)
        sbl = np.full(slots_tot, -2, np.int64)  # block of core's real slots
        # group boundaries of core's sorted edges: cumulative by (blk, bank)
        ccnt = cnt[c].reshape(-1)
        coff = np.concatenate([[0], np.cumsum(ccnt)])
        for b in range(nblk):
            for k in range(banks):
                i0 = coff[b * banks + k]
                i1 = coff[b * banks + k + 1]
                if i1 == i0:
                    continue
                o0 = blk_off[b, k]
                sidx[o0:o0 + i1 - i0] = s_c[i0:i1] - k * bs
                sdl[o0:o0 + i1 - i0] = d_c[i0:i1] % 128
                sbl[o0:o0 + i1 - i0] = b
        idx16[c] = _pack_idx16(sidx.astype(np.int16))
        # dstl columns per pair
        sdl2 = sdl.reshape(-1, 128)
        sbl2 = sbl.reshape(-1, 128)
        for j, (si, ch, b) in enumerate(pairs):
            ch_g = (segs[si]["slot0"] // 128) + ch
            col = np.where(sbl2[ch_g] == b, sdl2[ch_g], 999.0)
            dstl[c, :, j] = col

    # dinv per dst block [128, nblk]
    dinv_dst = np.ones((ncores, 128, nblk), dtype=np.float32)
    for c in range(ncores):
        v = dinv[c * npc:(c + 1) * npc]
        vp = np.ones(nblk * 128, dtype=np.float32)
        vp[:npc] = v
        dinv_dst[c] = vp.reshape(nblk, 128).T

    return dict(idx16=idx16, dstl=dstl, segs=segs, pairs=pairs,
                pair_start=pair_start, pair_stop=pair_stop,
                slots_tot=slots_tot, npairs=npairs,
                dinv_dst=dinv_dst, dinv=dinv, nsg=nsg, nblk=nblk)


def _prep_decode(eli, n=N, ncores=NCORES, banks=BANKS):
    bs = n // banks
    ppc = eli.shape[1] // ncores
    ii = np.asarray(eli[0], dtype=np.int64)
    jj = np.asarray(eli[1], dtype=np.int64)
    gcnt = np.zeros((ncores, banks * banks), np.int64)
    per_core = []
    for c in range(ncores):
        i0 = ii[c * ppc:(c + 1) * ppc]
        j0 = jj[c * ppc:(c + 1) * ppc]
        gid = (i0 // bs) * banks + (j0 // bs)
        order = np.argsort(gid, kind="stable")
        per_core.append((i0[order], j0[order], gid[order], order))
        gcnt[c] = np.bincount(gid, minlength=banks * banks)
    gmax = gcnt.max(axis=0)

    # common layout: per group, subcalls of <= DSUB slots (each 128-padded)
    subcalls = []  # (group, slot0, nslots)
    goff = np.zeros(banks * banks, np.int64)
    pos = 0
    for g in range(banks * banks):
        goff[g] = pos
        rem = int(gmax[g])
        while rem > 0:
            take = min(rem, DSUB)
            slots = -(-take // 128) * 128
            subcalls.append((g, pos, slots))
            pos += slots
            rem -= take
    slots_tot = pos
    ccpad = slots_tot // 128

    ii16 = np.zeros((ncores, 128, slots_tot // 16), np.int16)
    jj16 = np.zeros((ncores, 128, slots_tot // 16), np.int16)
    slotpos = np.zeros((ncores, ppc), np.int64)
    for c in range(ncores):
        i0, j0, gid, order = per_core[c]
        si = np.zeros(slots_tot, np.int64)
        sj = np.zeros(slots_tot, np.int64)
        cnts = gcnt[c]
        co = np.concatenate([[0], np.cumsum(cnts)])
        spos = np.zeros(ppc, np.int64)
        for g in range(banks * banks):
            a, bnd = co[g], co[g + 1]
            if bnd == a:
                continue
            # within-group slots may span multiple subcalls but those are
            # contiguous in the stream starting at goff[g]
            dstpos = goff[g] + np.arange(bnd - a)
            si[dstpos] = i0[a:bnd] % bs
            sj[dstpos] = j0[a:bnd] % bs
            spos[a:bnd] = dstpos
        slotpos[c][order] = spos
        ii16[c] = _pack_idx16(si.astype(np.int16))
        jj16[c] = _pack_idx16(sj.astype(np.int16))
    return dict(subcalls=subcalls, slots_tot=slots_tot, ccpad=ccpad,
                ii16=ii16, jj16=jj16, slotpos=slotpos, ppc=ppc)


def _build_layer(meta, n=N, npc=NPC, num_devices=NCORES, reps=1):
    """One GCN layer: tblT [128,n] bf16 -> out_rows [npc,128] + outT [128,npc]."""
    nblk = meta["nblk"]
    nsg = meta["nsg"]
    nt = (n + 127) // 128
    slots_tot = meta["slots_tot"]
    npairs = meta["npairs"]
    bs = n // BANKS
    nc = bacc.Bacc("TRN2", target_bir_lowering=False, debug=False,
                   num_devices=num_devices)
    tblT = nc.dram_tensor("tblT", [P, n], bf16, kind="ExternalInput").ap()
    W = nc.dram_tensor("W", [P, P], bf16, kind="ExternalInput").ap()
    brep = nc.dram_tensor("brep", [P, P], f32, kind="ExternalInput").ap()
    thr = nc.dram_tensor("thr", [P, 1], f32, kind="ExternalInput").ap()
    iota = nc.dram_tensor("iota", [P, P], f32, kind="ExternalInput").ap()
    idx16 = nc.dram_tensor("idx16", [P, slots_tot // 16], i16,
                           kind="ExternalInput").ap()
    dstl = nc.dram_tensor("dstl", [P, npairs], f32, kind="ExternalInput").ap()
    dd = nc.dram_tensor("dd", [P, nblk], f32, kind="ExternalInput").ap()
    sc2 = nc.dram_tensor("sc2", [P, nblk], f32, kind="ExternalInput").ap()
    out_rows = nc.dram_tensor("out_rows", [npc, P], bf16,
                              kind="ExternalOutput").ap()
    outT = nc.dram_tensor("outT", [P, npc], bf16, kind="ExternalOutput").ap()

    segs = meta["segs"]
    pairs = meta["pairs"]
    pair_start = meta["pair_start"]
    pair_stop = meta["pair_stop"]
    max_seg_ch = max((s["slots"] // 128 for s in segs), default=1)
    max_seg_pr = max((s["npairs"] for s in segs), default=1)

    WG = 8
    with tile.TileContext(nc) as tc:
        with (tc.tile_pool(name="const", bufs=1) as cpool,
              tc.tile_pool(name="xin", bufs=3) as xpool,
              tc.tile_pool(name="hs", bufs=3) as hpool,
              tc.tile_pool(name="g", bufs=2) as gpool,
              tc.tile_pool(name="m", bufs=2) as mpool,
              tc.tile_pool(name="ob", bufs=4) as opool,
              tc.tile_pool(name="ot", bufs=2) as otpool,
              tc.tile_pool(name="dram", bufs=1, space="DRAM") as dpool):
            htab = dpool.tile([n, P], bf16, name="htab")
            W_t = cpool.tile([P, P], bf16, name="W_t")
            nc.sync.dma_start(out=W_t[:], in_=W[:])
            brep_t = cpool.tile([P, P], f32, name="brep_t")
            nc.sync.dma_start(out=brep_t[:], in_=brep[:])
            thr_t = cpool.tile([P, 1], f32, name="thr_t")
            nc.sync.dma_start(out=thr_t[:], in_=thr[:])
            iota_t = cpool.tile([P, P], f32, name="iota_t")
            nc.sync.dma_start(out=iota_t[:], in_=iota[:])
            idx_t = cpool.tile([P, slots_tot // 16], i16, name="idx_t")
            nc.sync.dma_start(out=idx_t[:], in_=idx16[:])
            dstl_t = cpool.tile([P, npairs], f32, name="dstl_t")
            nc.sync.dma_start(out=dstl_t[:], in_=dstl[:])
            dd_t = cpool.tile([P, nblk], f32, name="dd_t")
            nc.sync.dma_start(out=dd_t[:], in_=dd[:])
            sc2_t = cpool.tile([P, nblk], f32, name="sc2_t")
            nc.sync.dma_start(out=sc2_t[:], in_=sc2[:])
            ident = cpool.tile([P, P], bf16, name="ident")
            make_identity(nc, ident[:])

            rep_cm = tc.For_i(0, reps, 1) if reps > 1 else None
            if rep_cm is not None:
                rep_cm.__enter__()

            # ---- dense: htab = cast_bf16(tblT.T @ W), 1024-row groups
            with tc.tile_pool(name="psA", bufs=3, space="PSUM") as psA:
                for g0 in range(0, nt, WG):
                    r0 = g0 * 128
                    grows = min(WG * 128, n - r0)
                    gtiles = (grows + 127) // 128
                    xg = xpool.tile([P, WG * P], bf16, name="xg", tag="xg")
                    nc.sync.dma_start(out=xg[:, :grows],
                                      in_=tblT[:, r0:r0+grows])
                    hs = hpool.tile([P, WG * P], bf16, name="hs", tag="hs")
                    for i in range(gtiles):
                        rows = min(128, grows - i * 128)
                        ps = psA.tile([P, P], f32, name="psA", tag="psA")
                        nc.tensor.matmul(ps[:rows, :],
                                         lhsT=xg[:, i*128:i*128+rows],
                                         rhs=W_t[:], start=True, stop=True)
                        nc.scalar.copy(out=hs[:rows, i*128:(i+1)*128],
                                       in_=ps[:rows, :])
                    ft = grows // 128
                    if ft:
                        nc.scalar.dma_start(
                            out=htab[r0:r0 + ft*128, :].rearrange(
                                "(g p) f -> p g f", p=P),
                            in_=hs[:, :ft*128].rearrange(
                                "p (g f) -> p g f", f=P))
                    rem = grows - ft * 128
                    if rem:
                        nc.scalar.dma_start(
                            out=htab[r0 + ft*128:r0 + grows, :],
                            in_=hs[:rem, ft*128:ft*128+P])

            # ---- aggregation
            with tc.tile_pool(name="psB", bufs=1, space="PSUM") as psB, \
                 tc.tile_pool(name="psT", bufs=2, space="PSUM") as psTp:
                ot = None
                for g in range(nsg):
                    b0 = g * SG
                    b1 = min(b0 + SG, nblk)
                    pstiles = {}
                    for bi in range(b1 - b0):
                        pstiles[bi] = psB.tile([P, P], f32, name="psB",
                                               tag=f"psB{bi}")
                    for k in range(BANKS):
                        seg = segs[g * BANKS + k]
                        if seg["slots"] == 0:
                            continue
                        nch = seg["slots"] // 128
                        npr = seg["npairs"]
                        g_t = gpool.tile([P, max_seg_ch * P], bf16,
                                         name="g", tag="g")
                        nc.gpsimd.dma_gather(
                            out_ap=g_t[:, :nch*P].rearrange(
                                "p (k f) -> p k f", f=P),
                            in_ap=htab[k*bs:(k+1)*bs, :],
                            idxs_ap=idx_t[:, seg["slot0"]//16:
                                          (seg["slot0"]+seg["slots"])//16],
                            num_idxs=seg["slots"],
                            num_idxs_reg=seg["slots"],
                            elem_size=P, single_packet=False)
                        M_t = mpool.tile([P, max_seg_pr * P], bf16,
                                         name="M", tag="M")
                        p0 = seg["pair0"]
                        nc.vector.tensor_tensor(
                            out=M_t[:, :npr*P].rearrange(
                                "p (k f) -> p k f", f=P),
                            in0=iota_t[:].rearrange("p (o f) -> p o f", o=1
                                                    ).broadcast_to([P, npr, P]),
                            in1=dstl_t[:, p0:p0+npr].rearrange(
                                "p (k o) -> p k o", o=1
                            ).broadcast_to([P, npr, P]),
                            op=mybir.AluOpType.is_equal)
                        for j, (ch, b) in enumerate(seg["pairs"]):
                            jj = p0 + j
                            nc.tensor.matmul(
                                pstiles[b - b0][:],
                                lhsT=M_t[:, j*P:(j+1)*P],
                                rhs=g_t[:, ch*P:(ch+1)*P],
                                start=bool(pair_start[jj]),
                                stop=bool(pair_stop[jj]))
                    for bi in range(b1 - b0):
                        b = b0 + bi
                        rows = min(128, npc - b * 128)
                        ps = pstiles[bi]
                        ob = opool.tile([P, P], f32, name="ob", tag="ob")
                        nc.vector.tensor_scalar(
                            out=ob[:], in0=ps[:], scalar1=dd_t[:, b:b+1],
                            scalar2=None, op0=mybir.AluOpType.mult)
                        nc.vector.tensor_tensor(out=ob[:], in0=ob[:],
                                                in1=brep_t[:],
                                                op=mybir.AluOpType.add)
                        orow = opool.tile([P, P], bf16, name="orow",
                                          tag="orow")
                        nc.vector.tensor_scalar(
                            out=orow[:], in0=ob[:], scalar1=thr_t[:, :1],
                            scalar2=sc2_t[:, b:b+1],
                            op0=mybir.AluOpType.max,
                            op1=mybir.AluOpType.mult)
                        nc.sync.dma_start(out=out_rows[b*128:b*128+rows, :],
                                          in_=orow[:rows, :])
                        if b % WG == 0:
                            ot = otpool.tile([P, WG * P], bf16, name="ot",
                                             tag="ot")
                        pst = psTp.tile([P, P], bf16, name="psT", tag="psT")
                        nc.tensor.transpose(out=pst[:, :rows],
                                            in_=orow[:rows, :],
                                            identity=ident[:rows, :rows])
                        off = (b % WG) * 128
                        nc.scalar.copy(out=ot[:, off:off+rows],
                                       in_=pst[:, :rows])
                        if b % WG == WG - 1 or b == nblk - 1:
                            bb0 = b - (b % WG)
                            cw = b * 128 + rows - bb0 * 128
                            nc.scalar.dma_start(
                                out=outT[:, bb0*128:bb0*128+cw],
                                in_=ot[:, :cw])

            if rep_cm is not None:
                rep_cm.__exit__(None, None, None)
    nc.compile()
    return nc


def _build_decode(dmeta, n=N, num_devices=NCORES, reps=1):
    """Decode: o[p, c] = sum_f z[ii[...], f] * z[jj[...], f] per slot."""
    bs = n // BANKS
    slots_tot = dmeta["slots_tot"]
    ccpad = dmeta["ccpad"]
    subcalls = dmeta["subcalls"]
    max_ch = max(s[2] for s in subcalls) // 128
    nc = bacc.Bacc("TRN2", target_bir_lowering=False, debug=False,
                   num_devices=num_devices)
    z = nc.dram_tensor("z", [n, P], bf16, kind="ExternalInput").ap()
    ii = nc.dram_tensor("ii", [P, slots_tot // 16], i16,
                        kind="ExternalInput").ap()
    jj = nc.dram_tensor("jj", [P, slots_tot // 16], i16,
                        kind="ExternalInput").ap()
    o = nc.dram_tensor("o", [P, ccpad], f32, kind="ExternalOutput").ap()
    with tile.TileContext(nc) as tc:
        with (tc.tile_pool(name="const", bufs=1) as cpool,
              tc.tile_pool(name="gath", bufs=2) as gpool,
              tc.tile_pool(name="pr", bufs=2) as ppool):
            ii_t = cpool.tile([P, slots_tot // 16], i16, name="ii_t")
            nc.sync.dma_start(out=ii_t[:], in_=ii[:])
            jj_t = cpool.tile([P, slots_tot // 16], i16, name="jj_t")
            nc.sync.dma_start(out=jj_t[:], in_=jj[:])
            oc = cpool.tile([P, ccpad], f32, name="oc")
            rep_cm = tc.For_i(0, reps, 1) if reps > 1 else None
            if rep_cm is not None:
                rep_cm.__enter__()
            for grp, slot0, nslots in subcalls:
                bi, bj = grp // BANKS, grp % BANKS
                nch = nslots // 128
                c0 = slot0 // 128
                gi = gpool.tile([P, max_ch * P], bf16, name="gi", tag="gi")
                nc.gpsimd.dma_gather(
                    out_ap=gi[:, :nch*P].rearrange("p (k f) -> p k f", f=P),
                    in_ap=z[bi*bs:(bi+1)*bs, :],
                    idxs_ap=ii_t[:, slot0//16:(slot0+nslots)//16],
                    num_idxs=nslots, num_idxs_reg=nslots, elem_size=P,
                    single_packet=False)
                gj = gpool.tile([P, max_ch * P], bf16, name="gj", tag="gj")
                nc.gpsimd.dma_gather(
                    out_ap=gj[:, :nch*P].rearrange("p (k f) -> p k f", f=P),
                    in_ap=z[bj*bs:(bj+1)*bs, :],
                    idxs_ap=jj_t[:, slot0//16:(slot0+nslots)//16],
                    num_idxs=nslots, num_idxs_reg=nslots, elem_size=P,
                    single_packet=False)
                pr = ppool.tile([P, max_ch * OUT], bf16, name="pr", tag="pr")
                nc.vector.tensor_tensor(
                    out=pr[:, :nch*OUT].rearrange("p (k f) -> p k f", f=OUT),
                    in0=gi[:, :nch*P].rearrange("p (k f) -> p k f",
                                                f=P)[:, :, :OUT],
                    in1=gj[:, :nch*P].rearrange("p (k f) -> p k f",
                                                f=P)[:, :, :OUT],
                    op=mybir.AluOpType.mult)
                nc.vector.tensor_reduce(
                    out=oc[:, c0:c0+nch],
                    in_=pr[:, :nch*OUT].rearrange("p (k f) -> p k f", f=OUT),
                    axis=mybir.AxisListType.X, op=mybir.AluOpType.add)
            if rep_cm is not None:
                rep_cm.__exit__(None, None, None)
            nc.sync.dma_start(out=o[:], in_=oc[:])
    nc.compile()
    return nc


def _get_programs(meta, dmeta):
    key = ("progs", meta["slots_tot"], meta["npairs"], dmeta["slots_tot"])
    if key not in _prog_cache:
        _prog_cache[key] = (_build_layer(meta), _build_decode(dmeta))
    return _prog_cache[key]


def _layer_maps(meta, tblT, Wv, brv, thv, sc2v, iota, ncores=NCORES):
    return [
        {"tblT": tblT, "W": Wv, "brep": brv, "thr": thv, "iota": iota,
         "idx16": meta["idx16"][c], "dstl": meta["dstl"][c],
         "dd": meta["dinv_dst"][c],
         "sc2": meta["dinv_dst"][c] if sc2v is None else sc2v}
        for c in range(ncores)
    ]


def kernel(x, W1, b1, W2, b2, edge_index, edge_label_idx):
    x = np.asarray(x, dtype=np.float32)
    W1 = np.asarray(W1, dtype=np.float32)
    b1 = np.asarray(b1, dtype=np.float32)
    W2 = np.asarray(W2, dtype=np.float32)
    b2 = np.asarray(b2, dtype=np.float32)
    eidx = np.asarray(edge_index)
    eli = np.asarray(edge_label_idx)

    meta = _prep(eidx)
    dmeta = _prep_decode(eli)
    nc_layer, nc_dec = _get_programs(meta, dmeta)

    dinv = meta["dinv"]
    iota = np.broadcast_to(np.arange(P, dtype=np.float32)[None, :],
                           (P, P)).copy()
    xT1 = np.ascontiguousarray((x * dinv[:, None]).T).astype(bfloat16)
    W1b = W1.astype(bfloat16)
    W2p = np.zeros((P, P), np.float32)
    W2p[:, :OUT] = W2
    W2b = W2p.astype(bfloat16)
    b1rep = np.broadcast_to(b1[None, :], (P, P)).astype(np.float32).copy()
    b2rep = np.zeros((P, P), np.float32)
    b2rep[:, :OUT] = b2[None, :]
    thr_relu = np.zeros((P, 1), np.float32)
    thr_id = np.full((P, 1), -1e30, np.float32)
    ones_sc = np.ones((P, NBLK), np.float32)

    core_ids = list(range(NCORES))
    res1 = run_bass_kernel_spmd(
        nc_layer, _layer_maps(meta, xT1, W1b, b1rep, thr_relu, None, iota),
        core_ids)
    h1T = np.concatenate([res1.results[c]["outT"] for c in range(NCORES)],
                         axis=1)
    res2 = run_bass_kernel_spmd(
        nc_layer, _layer_maps(meta, h1T, W2b, b2rep, thr_id, ones_sc, iota),
        core_ids)
    z = np.ascontiguousarray(np.concatenate(
        [res2.results[c]["out_rows"] for c in range(NCORES)], axis=0))
    # decode
    dec_maps = [{"z": z, "ii": dmeta["ii16"][c], "jj": dmeta["jj16"][c]}
                for c in range(NCORES)]
    res3 = run_bass_kernel_spmd(nc_dec, dec_maps, core_ids)
    outs = []
    for c in range(NCORES):
        slotvec = np.asarray(res3.results[c]["o"]).T.reshape(-1)
        outs.append(slotvec[dmeta["slotpos"][c]])
    return np.concatenate(outs).astype(np.float32)
